# revision 72
# baseline (speedup 1.0000x reference)
"""Trainium2 Bass kernel for nn_BioV_19748259627109.

Pipeline per core (data-parallel over batch B=8, one sample per core):
  S1  spatial 3x3 conv (1->3ch) as PE band-matmuls over H, f32r; silu -> bf16
  EX  direct SBUF->SBUF transposed DMA [h,(t,w)] -> [(q,t),(c,hq,w)] bf16,
      streamed per (c, t-half) chunk behind S1 compute
  S2  temporal depthwise conv (7 taps) as bf16 tile-positioned PE matmuls
  S3  silu1 (scalar) + silu2 (scalar, accum sum_g) + squares (gpsimd,
      accum sum_g2) per chunk
  AR  AllReduce of 6 floats (batch-norm terms of SwitchNorm), triggered
      immediately after stats; cc DMAs issued from the scalar queue
  KV  kv_s via block-diag PE contraction over t; kv_t via DVE dot rows
  OUT rank-1 outer product At (x) As on DVE/GPSIMD in 8 chunks, DMA'd out

The final GainControl output factors exactly as out[c,t,s] = At[c,t]*As[c,s],
and SwitchNorm is an affine per (b,c) that commutes with the kv contractions,
so the normalized tensor xn is never materialized.  Softmax exps skip the
max-subtraction: arguments are O(5) (normalized inputs times unit-norm
weights), far inside f32 range.
"""
import sys
if '/opt/trn_rl_repo' not in sys.path:
    sys.path.insert(0, '/opt/trn_rl_repo')

import numpy as np
from concourse import bass, bacc, tile, mybir

F32 = mybir.dt.float32
F32R = mybir.dt.float32r
BF16 = mybir.dt.bfloat16
BF16_NP = mybir.dt.np(BF16)
ALU = mybir.AluOpType
AFT = mybir.ActivationFunctionType
AXT = mybir.AxisListType

N_CORES = 8
B, T, H, W = 8, 32, 128, 128
C = 3
NTOT = float(T * H * W)
EPS = 1e-5


def _host_constants(inputs):
    w_s = np.asarray(inputs['w_spatial'], np.float32)     # (3,1,3,3)
    b_s = np.asarray(inputs['b_spatial'], np.float32)
    w_t = np.asarray(inputs['w_temporal'], np.float32)    # (3,1,7,1)
    b_t = np.asarray(inputs['b_temporal'], np.float32)
    sn_w = np.asarray(inputs['sn_weight'], np.float32).reshape(3)
    sn_b = np.asarray(inputs['sn_bias'], np.float32).reshape(3)
    mwr = np.asarray(inputs['mean_weight'], np.float32)
    vwr = np.asarray(inputs['var_weight'], np.float32)
    mw = np.exp(mwr - mwr.max()); mw = mw / mw.sum()
    vw = np.exp(vwr - vwr.max()); vw = vw / vw.sum()
    wkvs = np.asarray(inputs['w_kv_s'], np.float32)       # (2,32)
    wkvt = np.asarray(inputs['w_kv_t'], np.float32)       # (2,16384)

    # bandW[h_in, c, dx, h_out] = w_s[c,0,h_in-h_out+1,dx]
    hi = np.arange(128)[:, None]
    ho = np.arange(128)[None, :]
    dy = hi - ho + 1
    bandw = np.zeros((128, 3, 3, 128), np.float32)
    for c in range(3):
        for dx in range(3):
            m = np.where((dy >= 0) & (dy <= 2), w_s[c, 0, np.clip(dy, 0, 2), dx], 0.0)
            bandw[:, c, dx, :] = m.astype(np.float32)
    bandw = bandw.astype(BF16_NP)

    # bandT[(q,t_in), c, t_out] replicated over q
    ti = np.arange(32)[:, None]
    to = np.arange(32)[None, :]
    kk = ti - to + 3
    bandt32 = np.zeros((32, 3, 32), np.float32)
    for c in range(3):
        bandt32[:, c, :] = np.where((kk >= 0) & (kk <= 6), w_t[c, 0, np.clip(kk, 0, 6), 0], 0.0)
    # block-diagonal over quarters: [(q,t_in), c, (q0,t_out)]
    bandt = np.zeros((128, 3, 128), np.float32)
    for q in range(4):
        bandt[32 * q:32 * q + 32, :, 32 * q:32 * q + 32] = bandt32
    bandt = bandt.astype(BF16_NP)

    # kv_s lhsT [(q,t)=128, (o,q0)=8] -- o-major so evac rows are contiguous
    kvs_lhst = np.zeros((128, 8), np.float32)
    for q in range(4):
        for t in range(32):
            for o in range(2):
                kvs_lhst[q * 32 + t, o * 4 + q] = wkvs[o, t]
    kvs_lhst = kvs_lhst.astype(BF16_NP)

    qsum = np.zeros((128, 32), np.float32)
    qsum[np.arange(128), np.arange(128) % 32] = 1.0

    wkvt4 = wkvt.reshape(2, 4, 32, 128).transpose(1, 0, 2, 3).astype(BF16_NP)[None]  # (1,q,o,hq,w)

    ws_sum = wkvs.sum(axis=1)   # (2,)
    wt_sum = wkvt.sum(axis=1)   # (2,)
    # crow layout: [0:3] sn_w, [3:6] sn_b, [6:12] Ws[o] in (c,o) order,
    # [12:18] Wt[o] in (o,c) order
    crow = np.zeros((1, 32), np.float32)
    crow[0, 0:3] = sn_w
    crow[0, 3:6] = sn_b
    crow[0, 6:12] = np.tile(ws_sum, 3)                    # (c,o): Ws0,Ws1 x3
    crow[0, 12:18] = np.repeat(wt_sum, 3)                 # (o,c): Wt0 x3, Wt1 x3
    scal = dict(
        b_s=[float(v) for v in b_s], b_t=[float(v) for v in b_t],
        mw=[float(v) for v in mw], vw=[float(v) for v in vw],
    )
    return dict(bandw=bandw, bandt=bandt, kvs_lhst=kvs_lhst, qsum=qsum,
                wkvt4=wkvt4, crow=crow, scal=scal)


def build_program(scal, no_cc=False, use_rdma=True):
    """Builds the SPMD Bass program. scal: dict of baked python-float constants."""
    nc = bacc.Bacc("TRN2", target_bir_lowering=False, debug=False,
                   num_devices=N_CORES)

    xin = nc.dram_tensor("xin", [128, 32, 130], BF16, kind="ExternalInput")
    bandw_d = nc.dram_tensor("bandw", [128, 3, 3, 128], BF16, kind="ExternalInput")
    bandt_d = nc.dram_tensor("bandt", [128, 3, 128], BF16, kind="ExternalInput")
    kvsl_d = nc.dram_tensor("kvs_lhst", [128, 8], BF16, kind="ExternalInput")
    qsum_d = nc.dram_tensor("qsum", [128, 32], F32, kind="ExternalInput")
    wkvt_d = nc.dram_tensor("wkvt4", [1, 4, 2, 32, 128], BF16, kind="ExternalInput")
    crow_d = nc.dram_tensor("crow", [1, 32], F32, kind="ExternalInput")
    out_d = nc.dram_tensor("out", [3, 32, 128, 128], F32, kind="ExternalOutput")

    b_s, b_t = scal['b_s'], scal['b_t']
    mw, vw = scal['mw'], scal['vw']

    with tile.TileContext(nc) as tc:
        with (
            tc.tile_pool(name="const", bufs=1) as cpool,
            tc.tile_pool(name="big", bufs=1) as bigp,
            tc.tile_pool(name="work", bufs=2) as wpool,
            tc.tile_pool(name="outer", bufs=4) as opool,
            tc.tile_pool(name="psum", bufs=2, space="PSUM") as pp,
            tc.tile_pool(name="dram", bufs=1, space="DRAM") as dram,
        ):
            # ---- constant + input loads ----
            bandw_sb = cpool.tile([128, 3, 3, 128], BF16)
            nc.sync.dma_start(bandw_sb[:], bandw_d[:])
            x_sb = bigp.tile([128, 32, 130], BF16, tag="xbig")
            nc.sync.dma_start(x_sb[:, 0:16, :], xin[:, 0:16, :])
            nc.sync.dma_start(x_sb[:, 16:32, :], xin[:, 16:32, :])
            bandt_sb = cpool.tile([128, 3, 128], BF16)
            nc.sync.dma_start(bandt_sb[:], bandt_d[:])
            kvsl_sb = cpool.tile([128, 8], BF16)
            nc.sync.dma_start(kvsl_sb[:], kvsl_d[:])
            qsum_sb = cpool.tile([128, 32], F32)
            nc.sync.dma_start(qsum_sb[:], qsum_d[:])
            crow_sb = cpool.tile([1, 32], F32)
            nc.sync.dma_start(crow_sb[:], crow_d[:])
            bvals = cpool.tile([128, 8], F32)
            for c in range(3):
                nc.vector.memset(bvals[:, c:c + 1], b_s[c])
                nc.vector.memset(bvals[:, 3 + c:4 + c], b_t[c])
            nc.vector.memset(bvals[:, 6:7], EPS)
            nc.vector.memset(bvals[:, 7:8], 1.0)          # ones column
            ones_row = cpool.tile([1, 128], F32)
            nc.vector.memset(ones_row[:], 1.0)
            invn_col = cpool.tile([128, 1], F32)
            nc.vector.memset(invn_col[:], 1.0 / NTOT)
            wkvt_sb = cpool.tile([128, 2, 32, 128], BF16)
            for q in range(4):
                nc.sync.dma_start(
                    wkvt_sb[32 * q:32 * q + 32, :, :, :],
                    wkvt_d[0, q].unsqueeze(0).broadcast_to([32, 2, 32, 128]),
                )

            yB = bigp.tile([128, 3, 32, 128], BF16)    # [(q,t), c, hq, w]
            gB = bigp.tile([128, 3, 32, 128], BF16)
            ydram = dram.tile([3, 128, 32, 128], BF16)   # [c, h, t, w]

            # dummy collective at t~0: absorbs the framework barrier and the
            # cold CC-stream trigger delay so the real AllReduce starts
            # ~1us after its trigger instead of ~11.5us
            warm_in = dram.tile([1, 2], F32)
            warm_out = dram.tile([1, 2], F32)
            wrow = cpool.tile([1, 2], F32)
            nc.vector.memset(wrow[:], 0.0)
            nc.scalar.dma_start(warm_in[:], wrow[:])
            nc.gpsimd.collective_compute(
                "AllReduce", ALU.add,
                replica_groups=[list(range(N_CORES))],
                ins=[warm_in.opt()], outs=[warm_out.opt()])

            # ---- S1: spatial conv + silu + streamed DRAM-bounce exchange ----
            for c in range(3):
                for half in range(2):
                    t0 = 16 * half
                    ps = pp.tile([128, 2048], F32, tag="mm")
                    for j in range(4):
                        for dx in range(3):
                            nc.tensor.matmul(
                                ps[:, 512 * j:512 * (j + 1)],
                                lhsT=bandw_sb[:, c, dx, :],
                                rhs=x_sb[:, t0 + 4 * j:t0 + 4 * j + 4,
                                         dx:dx + 128],
                                start=(dx == 0), stop=(dx == 2),
                            )
                    ychunk = wpool.tile([128, 16, 128], BF16, tag="ychunk")
                    nc.scalar.activation(
                        ychunk[:].rearrange("p a b -> p (a b)"), ps[:],
                        AFT.Silu, bias=bvals[:, c:c + 1])
                    # write contiguous per partition: [c, h, t, w]
                    nc.sync.dma_start(ydram[c, :, t0:t0 + 16, :], ychunk[:])
                    # streamed readback: [c, h, t, w] -> yB[(q,t), c, hq, w]
                    for q in range(4):
                        nc.sync.dma_start(
                            yB[32 * q + t0:32 * q + t0 + 16, c, :, :],
                            ydram[c, 32 * q:32 * q + 32, t0:t0 + 16, :]
                                .transpose([1, 0, 2]),
                        )

            # ---- S2: temporal conv + fused silu1/silu2 + stats sidebands ----
            accs = cpool.tile([128, 12], F32)
            sq_scratch = cpool.tile([128, 2048], BF16)
            for c in range(3):
                for half in range(2):
                    hq0 = 16 * half
                    ps = pp.tile([128, 2048], F32, tag="mm")
                    for j in range(4):
                        nc.tensor.matmul(
                            ps[:, 512 * j:512 * (j + 1)],
                            lhsT=bandt_sb[:, c, :],
                            rhs=yB[:, c, hq0 + 4 * j:hq0 + 4 * j + 4, :],
                            start=True, stop=True,
                        )
                    zscr = wpool.tile([128, 2048], F32, tag="zchunk")
                    nc.scalar.activation(
                        zscr[:], ps[:],
                        AFT.Silu, bias=bvals[:, 3 + c:4 + c])
                    nc.scalar.activation(
                        gB[:, c, hq0:hq0 + 16, :].rearrange("p a b -> p (a b)"),
                        zscr[:], AFT.Silu,
                        accum_out=accs[:, 2 * c + half:2 * c + half + 1])
                    nc.vector.scalar_tensor_tensor(
                        sq_scratch[:],
                        gB[:, c, hq0:hq0 + 16, :].rearrange("p a b -> p (a b)"),
                        1.0,
                        gB[:, c, hq0:hq0 + 16, :].rearrange("p a b -> p (a b)"),
                        ALU.mult, ALU.mult,
                        accum_out=accs[:, 6 + 2 * c + half:7 + 2 * c + half])

            # ---- stats: PE partition-reduce + 3-op chain + AR kick ----
            # ddof correction N/(N-1) = 1.0000076 dropped (far below noise),
            # so temp = var_in + mean^2 = E2 exactly and the AR payload is
            # just (mean_in, E2) per channel.
            sc = cpool.tile([1, 32], F32)
            sc2 = cpool.tile([1, 32], F32)
            nc.vector.memset(sc[:, 22:24], 0.0)
            cc_in = dram.tile([1, 8], F32)
            cc_out = dram.tile([1, 8], F32)
            with tc.high_priority():
                # partition mean on the PE: (1/N)^T @ accs -> [1, 12]
                accr = pp.tile([1, 12], F32, tag="mm")
                nc.tensor.matmul(accr[:], lhsT=invn_col[:], rhs=accs[:],
                                 start=True, stop=True)
                accs_sb = cpool.tile([1, 12], F32)
                nc.vector.tensor_copy(accs_sb[:], accr[0:1, :])
                # halves-add: mean_in -> sc[16:19], E2 -> sc[19:22]
                nc.vector.tensor_add(sc[:, 16:19], accs_sb[0:1, 0:6:2],
                                     accs_sb[0:1, 1:6:2])
                nc.vector.tensor_add(sc[:, 19:22], accs_sb[0:1, 6:12:2],
                                     accs_sb[0:1, 7:12:2])

                nc.scalar.dma_start(cc_in[:], sc[:, 16:24])
                nc.gpsimd.collective_compute(
                    "AllReduce", ALU.add,
                    replica_groups=[list(range(N_CORES))],
                    ins=[cc_in.opt()], outs=[cc_out.opt()])
                nc.scalar.dma_start(sc[:, 24:32], cc_out[:])
                # local msq/var_in (needed only post-AR)
                nc.vector.tensor_mul(sc[:, 6:9], sc[:, 16:19], sc[:, 16:19])
                nc.vector.tensor_sub(sc[:, 9:12], sc[:, 19:22], sc[:, 6:9])
                # layer stats + the AR-independent halves of the mean/var
                # mixes, computed during the allreduce window:
                # m_loc [sc2 16:19] = mw0*mean_in + mw1*mean_ln
                # v_loc [sc2 20:23] = vw0*var_in + vw1*var_ln
                nc.vector.tensor_reduce(sc2[:, 12:13], sc[:, 16:19], AXT.X,
                                        ALU.add)
                nc.vector.tensor_scalar_mul(sc2[:, 12:13], sc2[:, 12:13],
                                            1.0 / 3)  # mean_ln
                nc.vector.tensor_reduce(sc2[:, 13:14], sc[:, 19:22], AXT.X,
                                        ALU.add)
                nc.vector.tensor_scalar_mul(sc2[:, 13:14], sc2[:, 13:14],
                                            1.0 / 3)  # Etemp_l
                nc.vector.tensor_mul(sc2[:, 14:15], sc2[:, 12:13],
                                     sc2[:, 12:13])
                nc.vector.tensor_sub(sc2[:, 15:16], sc2[:, 13:14],
                                     sc2[:, 14:15])   # var_ln
                nc.vector.tensor_scalar_mul(sc2[:, 26:27], sc2[:, 12:13],
                                            mw[1])
                nc.vector.tensor_scalar(sc2[:, 16:19], sc[:, 16:19], mw[0],
                                        sc2[:, 26:27], ALU.mult, ALU.add)
                nc.vector.tensor_scalar_mul(sc2[:, 27:28], sc2[:, 15:16],
                                            vw[1])
                nc.vector.tensor_scalar(sc2[:, 20:23], sc[:, 9:12], vw[0],
                                        sc2[:, 27:28], ALU.mult, ALU.add)
                # preload the Sqrt act table while the allreduce runs
                nc.scalar.activation(sc2[:, 31:32], sc[:, 19:20], AFT.Sqrt)

            # ---- kv_s contraction (PE) + evac + scatter ----
            kvs_tmp = bigp.tile([8, 4160], F32, tag="kvstmp")
            kvsA = cpool.tile([128, 3, 2, 128], F32)
            for c in range(3):
                for half in range(2):
                    hq0 = 16 * half
                    ps = pp.tile([8, 2048], F32, tag="mm")
                    for j in range(4):
                        nc.tensor.matmul(
                            ps[:, 512 * j:512 * (j + 1)],
                            lhsT=kvsl_sb[:],
                            rhs=gB[:, c, hq0 + 4 * j:hq0 + 4 * j + 4, :],
                            start=True, stop=True)
                    nc.scalar.copy(kvs_tmp[:, 2048 * half:2048 * (half + 1)],
                                   ps[:])
                for o in range(2):
                    nc.sync.dma_start(
                        kvsA[:, c, o, :],
                        kvs_tmp[4 * o:4 * o + 4, 0:4096],
                    )

            # ---- kv_t row dots (DVE, 1024-col pieces so stats ops can
            #      interleave without 4.4us stalls) ----
            kvt_acc = cpool.tile([128, 4, 6], F32)
            sq2 = wpool.tile([128, 1024], BF16, tag="sq")
            for o in range(2):
                for c in range(3):
                    for p in range(4):
                        nc.vector.scalar_tensor_tensor(
                            sq2[:],
                            gB[:, c].rearrange("p hq w -> p (hq w)")
                                [:, 1024 * p:1024 * (p + 1)],
                            1.0,
                            wkvt_sb[:, o].rearrange("p hq w -> p (hq w)")
                                [:, 1024 * p:1024 * (p + 1)],
                            ALU.mult, ALU.mult,
                            accum_out=kvt_acc[:, p, 3 * o + c:3 * o + c + 1])
            kvt6a = cpool.tile([128, 2, 6], F32)
            nc.vector.tensor_add(kvt6a[:, 0, :], kvt_acc[:, 0, :], kvt_acc[:, 1, :])
            nc.vector.tensor_add(kvt6a[:, 1, :], kvt_acc[:, 2, :], kvt_acc[:, 3, :])
            kvt6f = cpool.tile([128, 6], F32)
            nc.vector.tensor_add(kvt6f[:], kvt6a[:, 0, :], kvt6a[:, 1, :])
            ps_kvt = pp.tile([6, 32], F32, tag="mm")
            nc.tensor.matmul(ps_kvt[:], lhsT=kvt6f[:], rhs=qsum_sb[:],
                             start=True, stop=True)
            kvt6 = cpool.tile([6, 32], F32)
            nc.vector.tensor_copy(kvt6[:], ps_kvt[:])
            ktrow = cpool.tile([1, 192], F32)   # (o,c,t)
            nc.sync.dma_start(ktrow[:, 0:192], kvt6[:])

            # ---- post-AR math -> alpha/beta (batch terms only; the local
            #      halves were folded in during the allreduce window) ----
            nc.vector.tensor_scalar_mul(sc2[:, 0:3], sc[:, 24:27], 1.0 / B)  # mean_bn
            nc.vector.tensor_scalar_mul(sc2[:, 3:6], sc[:, 27:30], 1.0 / B)  # Etemp_b
            nc.vector.tensor_mul(sc2[:, 6:9], sc2[:, 0:3], sc2[:, 0:3])
            nc.vector.tensor_sub(sc2[:, 9:12], sc2[:, 3:6], sc2[:, 6:9])     # var_bn
            # mean [16:19] = m_loc + mw2*mean_bn; var [20:23] = v_loc + vw2*var_bn
            nc.vector.scalar_tensor_tensor(sc2[:, 16:19], sc2[:, 0:3], mw[2],
                                           sc2[:, 16:19], ALU.mult, ALU.add)
            nc.vector.scalar_tensor_tensor(sc2[:, 20:23], sc2[:, 9:12], vw[2],
                                           sc2[:, 20:23], ALU.mult, ALU.add)
            # rstd [28:31] = sqrt(1/(var + eps))
            nc.vector.tensor_scalar(sc2[:, 23:26], sc2[:, 20:23], EPS, None,
                                    ALU.add, ALU.bypass)
            nc.vector.reciprocal(sc2[:, 23:26], sc2[:, 23:26])
            nc.scalar.activation(sc2[:, 28:31], sc2[:, 23:26], AFT.Sqrt)
            # alpha [arow 0:3], beta [arow 3:6]
            arow = cpool.tile([1, 32], F32)
            nc.vector.tensor_mul(arow[:, 0:3], sc2[:, 28:31], crow_sb[:, 0:3])  # alpha
            nc.vector.tensor_mul(arow[:, 3:6], sc2[:, 16:19], arow[:, 0:3])
            nc.vector.tensor_sub(arow[:, 3:6], crow_sb[:, 3:6], arow[:, 3:6])  # beta
            # broadcast rows: alphao (c,o) [6:12], betaws (c,o) [12:18],
            # alphaoc (o,c) [18:24], betawt (o,c) [24:30]
            nc.vector.tensor_copy(
                arow[:, 6:12].rearrange("p (c o) -> p c o", c=3),
                arow[:, 0:3].unsqueeze(2).broadcast_to([1, 3, 2]))
            nc.vector.tensor_mul(
                arow[:, 12:18].rearrange("p (c o) -> p c o", c=3),
                arow[:, 3:6].unsqueeze(2).broadcast_to([1, 3, 2]),
                crow_sb[:, 6:12].rearrange("p (c o) -> p c o", c=3))
            nc.vector.tensor_copy(
                arow[:, 18:24].rearrange("p (o c) -> p o c", o=2),
                arow[:, 0:3].unsqueeze(1).broadcast_to([1, 2, 3]))
            nc.vector.tensor_mul(
                arow[:, 24:30].rearrange("p (o c) -> p o c", o=2),
                arow[:, 3:6].unsqueeze(1).broadcast_to([1, 2, 3]),
                crow_sb[:, 12:18].rearrange("p (o c) -> p o c", o=2))
            # broadcast to all partitions via PE: ones_row^T @ arow
            ab_rep = pp.tile([128, 12], F32, tag="mm")
            nc.tensor.matmul(ab_rep[:], lhsT=ones_row[:], rhs=arow[:, 6:18],
                             start=True, stop=True)

            # ---- As: affine + exp (no max-sub; args are O(5)) ----
            # k-lane affine first so the exps can start ASAP; the v-lane
            # affine runs on vector while the scalar engine does the exps.
            nc.vector.tensor_mul(
                kvsA[:, :, 0, :],
                kvsA[:, :, 0, :],
                ab_rep[:, 0:6].rearrange("p (c o) -> p c o", c=3)[:, :, 0:1]
                     .broadcast_to([128, 3, 128]))
            nc.vector.tensor_add(
                kvsA[:, :, 0, :],
                kvsA[:, :, 0, :],
                ab_rep[:, 6:12].rearrange("p (c o) -> p c o", c=3)[:, :, 0:1]
                     .broadcast_to([128, 3, 128]))
            red = cpool.tile([128, 16], F32)
            escr = cpool.tile([128, 3, 128], F32)
            for c in range(3):
                nc.scalar.activation(escr[:, c], kvsA[:, c, 0, :], AFT.Exp,
                                     accum_out=red[:, 9 + c:10 + c])
            # v-lane affine (overlaps the exps)
            nc.vector.tensor_mul(
                kvsA[:, :, 1, :],
                kvsA[:, :, 1, :],
                ab_rep[:, 0:6].rearrange("p (c o) -> p c o", c=3)[:, :, 1:2]
                     .broadcast_to([128, 3, 128]))
            nc.vector.tensor_add(
                kvsA[:, :, 1, :],
                kvsA[:, :, 1, :],
                ab_rep[:, 6:12].rearrange("p (c o) -> p c o", c=3)[:, :, 1:2]
                     .broadcast_to([128, 3, 128]))
            redr = pp.tile([1, 3], F32, tag="mm")
            nc.tensor.matmul(redr[:], lhsT=bvals[:, 7:8], rhs=red[:, 9:12],
                             start=True, stop=True)
            ehalf = cpool.tile([128, 3, 128], F32)
            nc.scalar.activation(ehalf[:], kvsA[:, :, 0, :], AFT.Exp, scale=0.5)
            # preload the Sqrt table for rs while vector computes prod;
            # reading ehalf forces this after the exps (else the Exp table
            # load evicts the preloaded Sqrt table)
            nc.scalar.activation(sc2[:, 30:31], ehalf[0:1, 0, 0:1], AFT.Sqrt)
            AsA = cpool.tile([128, 3, 1, 128], F32)
            nc.vector.tensor_mul(AsA[:, :, 0, :], ehalf[:], kvsA[:, :, 1, :])

            # ---- At: affine + exp over t (single lane) ----
            nc.vector.tensor_mul(
                ktrow[:].rearrange("p (oc t) -> p oc t", oc=6),
                ktrow[:].rearrange("p (oc t) -> p oc t", oc=6),
                arow[:, 18:24].unsqueeze(2).broadcast_to([1, 6, 32]))
            nc.vector.tensor_add(
                ktrow[:].rearrange("p (oc t) -> p oc t", oc=6),
                ktrow[:].rearrange("p (oc t) -> p oc t", oc=6),
                arow[:, 24:30].unsqueeze(2).broadcast_to([1, 6, 32]))
            trow = cpool.tile([1, 256], F32)
            # slots: efull [128:224], tsum [3:6], prod [9:12], rs [6:9],
            # ehalf [224:320 -> use 0..96 of trow2]
            nc.scalar.activation(trow[:, 128:224], ktrow[:, 0:96], AFT.Exp)
            nc.vector.tensor_reduce(
                trow[:, 3:6], trow[:, 128:224].rearrange("p (c t) -> p c t", c=3),
                AXT.X, ALU.add)
            # prod = sum_s * sum_t per c; rs = sqrt(1/prod)
            nc.vector.tensor_mul(trow[:, 9:12], trow[:, 3:6], redr[0:1, 0:3])
            nc.vector.reciprocal(trow[:, 9:12], trow[:, 9:12])
            nc.scalar.activation(trow[:, 6:9], trow[:, 9:12], AFT.Sqrt)
            trow2 = cpool.tile([1, 96], F32)
            nc.scalar.activation(trow2[:], ktrow[:, 0:96], AFT.Exp, scale=0.5)
            atrow = cpool.tile([1, 96], F32)
            nc.vector.tensor_mul(atrow[:], trow2[:], ktrow[:, 96:192])
            nc.vector.tensor_mul(
                atrow[:].rearrange("p (c t) -> p c t", c=3),
                atrow[:].rearrange("p (c t) -> p c t", c=3),
                trow[:, 6:9].unsqueeze(2).broadcast_to([1, 3, 32]))
            atrep = pp.tile([128, 96], F32, tag="mm")
            nc.tensor.matmul(atrep[:], lhsT=ones_row[:], rhs=atrow[:],
                             start=True, stop=True)

            # ---- outer product + output DMA (8 chunks of 4 t) ----
            # chunk 0 computes per-channel so its first DMA issues after a
            # 512-col multiply instead of the full 1536-col chunk
            for chunk in range(8):
                t0 = 4 * chunk
                ost = opool.tile([128, 3, 4, 128], F32, tag="ochunk")
                if chunk == 0:
                    for c in range(3):
                        nc.vector.tensor_tensor(
                            ost[:, c],
                            AsA[:, c].broadcast_to([128, 4, 128]),
                            atrep[:].rearrange("p (c t) -> p c t", c=3)
                                 .unsqueeze(3)[:, c, t0:t0 + 4, :]
                                 .broadcast_to([128, 4, 128]),
                            ALU.mult)
                        deng = nc.sync if c % 2 == 0 else nc.scalar
                        deng.dma_start(
                            out_d[c, t0:t0 + 4, :, :].transpose([1, 0, 2]),
                            ost[:, c])
                    continue
                nc.vector.tensor_tensor(
                    ost[:],
                    AsA[:].broadcast_to([128, 3, 4, 128]),
                    atrep[:].rearrange("p (c t) -> p c t", c=3).unsqueeze(3)
                         [:, :, t0:t0 + 4, :].broadcast_to([128, 3, 4, 128]),
                    ALU.mult)
                for c in range(3):
                    deng = nc.sync if (3 * chunk + c) % 2 == 0 else nc.scalar
                    deng.dma_start(
                        out_d[c, t0:t0 + 4, :, :].transpose([1, 0, 2]),
                        ost[:, c])

    nc.compile()
    return nc


def _in_maps(inputs, consts):
    x = np.asarray(inputs['x'], np.float32)
    maps = []
    for b in range(N_CORES):
        xp = np.zeros((128, 32, 130), BF16_NP)
        xp[:, :, 1:129] = x[b, 0].transpose(1, 0, 2).astype(BF16_NP)
        maps.append(dict(
            xin=xp, bandw=consts['bandw'], bandt=consts['bandt'],
            kvs_lhst=consts['kvs_lhst'], qsum=consts['qsum'],
            wkvt4=consts['wkvt4'], crow=consts['crow'],
        ))
    return maps


def kernel(**inputs) -> np.ndarray:
    from concourse.bass_utils import run_bass_kernel_spmd
    consts = _host_constants(inputs)
    nc = build_program(consts['scal'])
    maps = _in_maps(inputs, consts)
    res = run_bass_kernel_spmd(nc, maps, list(range(N_CORES)))
    out = np.stack([res.results[b]['out'] for b in range(N_CORES)], axis=0)
    return out.astype(np.float32)
